# revision 23
# baseline (speedup 1.0000x reference)
"""Trainium2 Bass kernel for a dense transformer block (B=2, S=2048, D=1024,
H=16, d_ff=4096), sharded over 8 NeuronCores.

Sharding: DP(2 groups over batch) x TP(4 cores over heads) for
LN1/QKV/attention/proj, pipelined per 512-token chunk with a per-chunk
bf16 ReduceScatter of the proj partials; then token-parallel MLP (each
core: 512 tokens, full MLP weights). Host assembles the 8 per-core
outputs.

v1 optimizations over the original baseline:
- LN gamma/beta folded into W_qkv / W_fc on the host; V-projection bias
  folded into the proj bias (softmax rows sum to 1), proj bias folded
  into the po copy as bias/4 (RS sums 4 partials).
- LayerNorm stats via bn_stats/bn_aggr (one DVE pass), rstd via
  sqrt + reciprocal_approx_fast (no DVE table reloads).
- Softmax normalization: reciprocal_approx_fast on the ones-column
  denominator + gpsimd partition_broadcast + single multiply (no psum
  copy, no bias add).
- ReduceScatter payload in bf16 (half the bytes).
- Residual + LN2 + h2T transpose pipelined per chunk (runs during the
  next chunk's attention); first half of fc1 runs under the last RS.
- LN transposes (xc^T @ diag(rstd)) in bf16 (1 cyc/row vs 4 for f32).
"""

from contextlib import ExitStack

import numpy as np

import concourse.bacc as bacc
import concourse.mybir as mybir
import concourse.tile as tile
from concourse.bass_utils import run_bass_kernel_spmd
from concourse.masks import make_identity

f32 = mybir.dt.float32
bf16 = mybir.dt.bfloat16
AF = mybir.ActivationFunctionType
OP = mybir.AluOpType

B = 2
S_FULL = 2048
D = 1024
H = 16
HD = 64
DFF_FULL = 4096
LN_EPS = 1e-5
N_CORES = 8
GROUP_FULL = 4
HPC = 4
DJ = D // 128
CS = 512


def build_nc(S=S_FULL, DFF=DFF_FULL, GROUP=GROUP_FULL, n_cores=N_CORES):
    at = bf16
    mt = bf16
    NCH = S // CS
    SL = S // GROUP
    SLT = SL // 128
    NF = DFF // 128
    KT = S // 128
    groups = [list(range(g * GROUP, (g + 1) * GROUP))
              for g in range(n_cores // GROUP)]

    nc = bacc.Bacc("TRN2", target_bir_lowering=False, debug=False,
                   num_devices=n_cores)

    def din(name, shape, dt=f32):
        return nc.dram_tensor(name, shape, dt, kind="ExternalInput").ap()

    x_d = din("x_b", [S, D])
    xo_d = din("x_own", [SL, D])
    wq_d = din("wq_m", [128, DJ, 256], at)
    wk_d = din("wk_m", [128, DJ, 256], at)
    wv_d = din("wv_m", [128, DJ, 256], at)
    bq_d = din("bq_m", [128, 2])
    bk_d = din("bk_m", [128, 2])
    wp_d = din("wproj_m", [128, 2, D], at)
    bpq_d = din("bprojq_m", [1, D])
    wfc_d = din("wfc_m", [NF, 128, DJ, 128], mt)
    bfc_d = din("bfc_m", [128, NF])
    wo_d = din("wout_m", [DFF, D], mt)
    bo_d = din("bout_m", [1, D], mt)
    out_d = nc.dram_tensor("out_s", [SL, D], f32, kind="ExternalOutput").ap()

    with tile.TileContext(nc) as tc, ExitStack() as st0:
        su = st0.enter_context(tc.tile_pool(name="setup", bufs=1))
        ws = st0.enter_context(tc.tile_pool(name="wstream", bufs=3))
        drp = st0.enter_context(tc.tile_pool(name="dram", bufs=1, space="DRAM"))

        cc_ins = [drp.tile([CS, D], at, name=f"cc_in{i}")
                  for i in range(NCH)]
        cc_outs = [drp.tile([128, D], at, name=f"cc_out{i}")
                   for i in range(NCH)]
        wu_in = drp.tile([4, 4], f32, name="wu_in")
        wu_out = drp.tile([16, 4], f32, name="wu_out")

        # warmup collective: syncs the cores before the pipeline so the
        # first real ReduceScatter doesn't absorb core-start skew
        with tc.tile_pool(name="wup", bufs=1) as wup:
            wt = wup.tile([4, 4], f32, name="wt")
            nc.vector.memset(wt[:], 1.0)
            nc.sync.dma_start(wu_in[:], wt[:])
            nc.gpsimd.collective_compute(
                "AllGather", OP.bypass, replica_groups=groups,
                ins=[wu_in[:].opt()], outs=[wu_out[:].opt()])

        ident = su.tile([128, 128], f32, name="ident")
        make_identity(nc, ident[:])
        negC = su.tile([128, 1], f32, name="negC")
        nc.vector.memset(negC[:], -4.0)
        epsb = su.tile([128, 1], f32, name="epsb")
        nc.vector.memset(epsb[:], LN_EPS)

        bq_sb = su.tile([128, 2], f32, name="bq_sb")
        nc.sync.dma_start(bq_sb[:], bq_d)
        bk_sb = su.tile([128, 2], f32, name="bk_sb")
        nc.sync.dma_start(bk_sb[:], bk_d)
        bfc_sb = su.tile([128, NF], f32, name="bfc_sb")
        nc.sync.dma_start(bfc_sb[:], bfc_d)
        bout_sb = su.tile([1, D], mt, name="bout_sb")
        nc.sync.dma_start(bout_sb[:], bo_d)
        ones1 = su.tile([1, 128], mt, name="ones1")
        nc.vector.memset(ones1[:], 1.0)

        bprojq_bc = su.tile([128, D], f32, name="bprojq_bc")
        with tc.tile_pool(name="tmpb", bufs=1) as tb:
            brow = tb.tile([1, D], f32, name="brow")
            nc.sync.dma_start(brow[:], bpq_d)
            nc.gpsimd.partition_broadcast(bprojq_bc[:], brow[:])

        # static causal masks for the diagonal key tiles: cmask[i][p, c] =
        # 1 if c - p >= i*128 else 0  (key-tile offset d0 = k0-q0 = i*128)
        cmask = su.tile([128, 4, 512], bf16, name="cmask")
        nc.vector.memset(cmask[:], 1.0)
        for i in range(4):
            nc.gpsimd.affine_select(
                out=cmask[:, i, :], in_=cmask[:, i, :],
                compare_op=OP.is_ge, fill=0.0, base=-i * 128,
                pattern=[[1, 512]], channel_multiplier=-1)

        # persistent attention-side tensors
        ap = st0.enter_context(tc.tile_pool(name="attn_per", bufs=1))
        Qt = ap.tile([128, 2, S], at, name="Qt")
        Kt = ap.tile([128, 2, S], at, name="Kt")
        yT = ap.tile([128, 2, S], at, name="yT")
        Vg = ap.tile([128, KT, HPC, 65], at, name="Vg")
        nc.vector.memset(Vg[:, :, :, 64:65], 1.0)
        Wp_sb = ap.tile([128, 2, D], at, name="Wp_sb")
        Wq_sb = ap.tile([128, DJ, 256], at, name="Wq_sb")
        Wk_sb = ap.tile([128, DJ, 256], at, name="Wk_sb")
        Wv_sb = ap.tile([128, DJ, 256], at, name="Wv_sb")

        # persistent MLP-side tensors
        xP = ap.tile([128, SLT, 2, 512], f32, name="xP")
        h2T = ap.tile([128, DJ, SL], mt, name="h2T")
        m1T = ap.tile([128, NF, SL], mt, name="m1T")

        with ExitStack() as st1:
            p1x = st1.enter_context(tc.tile_pool(name="p1x", bufs=4))
            p1xc = st1.enter_context(tc.tile_pool(name="p1xc", bufs=4))
            p1ht = st1.enter_context(tc.tile_pool(name="p1ht", bufs=2))
            p1d = st1.enter_context(tc.tile_pool(name="p1d", bufs=8))
            p1s = st1.enter_context(tc.tile_pool(name="p1s", bufs=8))
            p2e = st1.enter_context(tc.tile_pool(name="p2e", bufs=2))
            p2n = st1.enter_context(tc.tile_pool(name="p2n", bufs=2))
            p2o = st1.enter_context(tc.tile_pool(name="p2o", bufs=2))
            p4z = st1.enter_context(tc.tile_pool(name="p4z", bufs=2))
            p4xc = st1.enter_context(tc.tile_pool(name="p4xc", bufs=2))
            p4s = st1.enter_context(tc.tile_pool(name="p4s", bufs=8))
            pmm = st1.enter_context(tc.tile_pool(name="pmm", bufs=2,
                                                 space="PSUM"))
            pss = st1.enter_context(tc.tile_pool(name="pss", bufs=2,
                                                 space="PSUM"))
            p2y = st1.enter_context(tc.tile_pool(name="p2y", bufs=2,
                                                 space="PSUM"))

            x_cache = {}

            def load_x(ch):
                tiles = []
                for tl in range(4):
                    ti = ch * 4 + tl
                    xt = p1x.tile([128, 2, 512], f32, name="xt", tag="xt")
                    nc.sync.dma_start(
                        xt[:, 0, :], x_d[ti * 128:(ti + 1) * 128, 0:512])
                    nc.sync.dma_start(
                        xt[:, 1, :], x_d[ti * 128:(ti + 1) * 128, 512:1024])
                    tiles.append(xt)
                x_cache[ch] = tiles

            def emit_ln1_qkv(ch):
                # LayerNorm1 stats + centered x + diag(rstd), per token tile
                if ch not in x_cache:
                    load_x(ch)
                xts = x_cache.pop(ch)
                xcs, diags = [], []
                for tl in range(4):
                    xt = xts[tl]
                    bns = p1s.tile([128, 2, 6], f32, name="bns", tag="bns")
                    nc.vector.bn_stats(bns[:, 0, :], xt[:, 0, :])
                    nc.vector.bn_stats(bns[:, 1, :], xt[:, 1, :])
                    mv = p1s.tile([128, 2], f32, name="mv", tag="mv")
                    nc.vector.bn_aggr(mv[:], bns[:])
                    xc = p1xc.tile([128, 2, 512], at, name="xc", tag="xc")
                    nc.vector.tensor_scalar(
                        xc[:], xt[:], mv[:, 0:1], None, OP.subtract)
                    sd = p1s.tile([128, 1], f32, name="sd", tag="sd")
                    nc.scalar.activation(
                        sd[:], mv[:, 1:2], AF.Sqrt, bias=epsb[:], scale=1.0)
                    rstd = p1s.tile([128, 1], f32, name="rstd", tag="rstd")
                    nc.vector.reciprocal_approx_fast(rstd[:], sd[:])
                    dg = p1d.tile([128, 128], at, name="dg", tag="dg")
                    nc.vector.tensor_scalar_mul(dg[:], ident[:], rstd[:])
                    xcs.append(xc)
                    diags.append(dg)

                # h^T (normalized-x transpose) via diag matmuls; gamma/beta
                # already folded into W_qkv on the host.
                hT = p1ht.tile([128, DJ, CS], at, name="hT", tag="hT")
                for j in range(DJ):
                    ptt = pmm.tile([128, 512], f32, name="ptt", tag="mm")
                    for tl in range(4):
                        nc.tensor.matmul(
                            ptt[:, tl * 128:(tl + 1) * 128],
                            xcs[tl][:, j // 4, (j % 4) * 128:(j % 4) * 128 + 128],
                            diags[tl][:], start=True, stop=True)
                    nc.vector.tensor_copy(hT[:, j, :], ptt[:])

                # QKV
                for hp in range(2):
                    psq = pmm.tile([128, 512], f32, name="psq", tag="mm")
                    for j in range(DJ):
                        nc.tensor.matmul(
                            psq[:], Wq_sb[:, j, hp * 128:(hp + 1) * 128],
                            hT[:, j, :], start=(j == 0), stop=(j == DJ - 1))
                    nc.vector.tensor_scalar(
                        Qt[:, hp, ch * CS:(ch + 1) * CS], psq[:],
                        bq_sb[:, hp:hp + 1], None, OP.add)
                    psk = pmm.tile([128, 512], f32, name="psk", tag="mm")
                    for j in range(DJ):
                        nc.tensor.matmul(
                            psk[:], Wk_sb[:, j, hp * 128:(hp + 1) * 128],
                            hT[:, j, :], start=(j == 0), stop=(j == DJ - 1))
                    nc.vector.tensor_scalar(
                        Kt[:, hp, ch * CS:(ch + 1) * CS], psk[:],
                        bk_sb[:, hp:hp + 1], None, OP.add)
                for tl in range(4):
                    ti = ch * 4 + tl
                    psv = pmm.tile([128, 512], f32, name="psv", tag="mm")
                    for j in range(DJ):
                        nc.tensor.matmul(
                            psv[:, 0:256],
                            hT[:, j, tl * 128:(tl + 1) * 128],
                            Wv_sb[:, j, :], start=(j == 0),
                            stop=(j == DJ - 1))
                    for h in range(HPC):
                        nc.vector.tensor_copy(
                            Vg[:, ti, h, 0:64],
                            psv[:, h * 64:(h + 1) * 64])

            def emit_attn(ch):
                q0 = ch * CS
                nkj = (q0 + CS) // 128
                # h2=1 heads first: their yT writes bounce through an
                # SBUF DMA, so get them off the proj critical path
                for hp in range(2):
                    for h2 in (1, 0):
                        h = hp * 2 + h2
                        psy = p2y.tile([128, CS], f32, name="psy", tag="psy")
                        first = True
                        for g0 in range(0, nkj, 2):
                            psc = pss.tile([128, 1024], f32, name="psc",
                                           tag="pss")
                            for kk in range(2):
                                kjt = g0 + kk
                                nc.tensor.matmul(
                                    psc[:, kk * 512:(kk + 1) * 512],
                                    Kt[h2 * 64:(h2 + 1) * 64, hp,
                                       kjt * 128:(kjt + 1) * 128],
                                    Qt[h2 * 64:(h2 + 1) * 64, hp,
                                       q0:q0 + CS],
                                    start=True, stop=True)
                            es = p2e.tile([128, 1024], at, name="es",
                                          tag="es")
                            nc.scalar.activation(
                                es[:], psc[:], AF.Exp, bias=negC[:],
                                scale=0.125)
                            for kk in range(2):
                                kjt = g0 + kk
                                k0 = kjt * 128
                                if k0 >= q0:
                                    nc.vector.tensor_tensor(
                                        es[:, kk * 512:(kk + 1) * 512],
                                        es[:, kk * 512:(kk + 1) * 512],
                                        cmask[:, (k0 - q0) // 128, :],
                                        OP.mult)
                                nc.tensor.matmul(
                                    psy[0:65, :], Vg[:, kjt, h, :],
                                    es[:, kk * 512:(kk + 1) * 512],
                                    start=first, stop=(kjt == nkj - 1))
                                first = False
                        # normalize by the ones-column denominator
                        # (custom DVE needs partition-base-0 SBUF operands)
                        den = p2n.tile([1, 512], f32, name="den", tag="den")
                        nc.vector.tensor_copy(den[:], psy[64:65, :])
                        ivf = p2n.tile([1, 512], f32, name="ivf", tag="ivf")
                        nc.vector.reciprocal_approx_fast(ivf[:], den[:])
                        ivh = p2n.tile([1, 512], at, name="ivh", tag="ivh")
                        nc.vector.tensor_copy(ivh[:], ivf[:])
                        # broadcast across 64 partitions via a K=1 matmul
                        # (keeps gpsimd free for the collectives), then
                        # bounce to SBUF (only one PSUM operand per DVE op)
                        psb = pmm.tile([128, 512], f32, name="psb", tag="mm")
                        nc.tensor.matmul(psb[0:64, :], ones1[:, 0:64],
                                         ivh[:], start=True, stop=True)
                        bcst = p2n.tile([64, 512], f32, name="bcst",
                                        tag="bcst")
                        nc.vector.tensor_copy(bcst[:], psb[0:64, :])
                        if h2 == 0:
                            nc.vector.tensor_tensor(
                                yT[0:64, hp, q0:q0 + CS], psy[0:64, :],
                                bcst[:], OP.mult)
                        else:
                            st2 = p2n.tile([64, 512], at, name="st2",
                                           tag="st2")
                            nc.vector.tensor_tensor(
                                st2[:], psy[0:64, :], bcst[:], OP.mult)
                            nc.sync.dma_start(
                                yT[64:128, hp, q0:q0 + CS], st2[:])

            def emit_proj_rs(ch):
                for tl in range(4):
                    ti = ch * 4 + tl
                    for n in range(2):
                        psp = pmm.tile([128, 512], f32, name="psp", tag="mm")
                        for hp in range(2):
                            nc.tensor.matmul(
                                psp[:],
                                yT[:, hp, ti * 128:(ti + 1) * 128],
                                Wp_sb[:, hp, n * 512:(n + 1) * 512],
                                start=(hp == 0), stop=(hp == 1))
                        po = p2o.tile([128, 512], at, name="po", tag="po")
                        nc.vector.tensor_tensor(
                            po[:], psp[:],
                            bprojq_bc[:, n * 512:(n + 1) * 512], OP.add)
                        nc.sync.dma_start(
                            cc_ins[ch][tl * 128:(tl + 1) * 128,
                                       n * 512:(n + 1) * 512], po[:])
                nc.gpsimd.collective_compute(
                    "ReduceScatter", OP.add, replica_groups=groups,
                    ins=[cc_ins[ch][:].opt()],
                    outs=[cc_outs[ch][:].opt()])

            def emit_resid_ln2(pch):
                zt = p4z.tile([128, 2, 512], at, name="zt", tag="zt")
                nc.sync.dma_start(zt[:, 0, :], cc_outs[pch][:, 0:512])
                nc.sync.dma_start(zt[:, 1, :], cc_outs[pch][:, 512:1024])
                xre = p4z.tile([128, 2, 512], f32, name="xre", tag="xre")
                nc.sync.dma_start(
                    xre[:, 0, :], xo_d[pch * 128:(pch + 1) * 128, 0:512])
                nc.sync.dma_start(
                    xre[:, 1, :], xo_d[pch * 128:(pch + 1) * 128, 512:1024])
                xPc = xP[:, pch, :, :]
                nc.vector.tensor_tensor(xPc, zt[:], xre[:], OP.add)
                bns2 = p4s.tile([128, 2, 6], f32, name="bns2", tag="bns2")
                nc.vector.bn_stats(bns2[:, 0, :], xPc[:, 0, :])
                nc.vector.bn_stats(bns2[:, 1, :], xPc[:, 1, :])
                mv2 = p4s.tile([128, 2], f32, name="mv2", tag="mv2")
                nc.vector.bn_aggr(mv2[:], bns2[:])
                xc2 = p4xc.tile([128, 2, 512], at, name="xc2", tag="xc2")
                nc.vector.tensor_scalar(
                    xc2[:], xPc, mv2[:, 0:1], None, OP.subtract)
                sd2 = p4s.tile([128, 1], f32, name="sd2", tag="sd2")
                nc.scalar.activation(
                    sd2[:], mv2[:, 1:2], AF.Sqrt, bias=epsb[:], scale=1.0)
                rstd2 = p4s.tile([128, 1], f32, name="rstd2", tag="rstd2")
                nc.vector.reciprocal_approx_fast(rstd2[:], sd2[:])
                dg2 = p1d.tile([128, 128], at, name="dg2", tag="dg")
                nc.vector.tensor_scalar_mul(dg2[:], ident[:], rstd2[:])
                for jg in range(2):
                    pt2 = pmm.tile([128, 512], f32, name="pt2", tag="mm")
                    for jj in range(4):
                        j = jg * 4 + jj
                        nc.tensor.matmul(
                            pt2[:, jj * 128:(jj + 1) * 128],
                            xc2[:, j // 4, (j % 4) * 128:(j % 4) * 128 + 128],
                            dg2[:], start=True, stop=True)
                    nc.vector.tensor_copy(
                        h2T[:, jg * 4:(jg + 1) * 4,
                            pch * 128:(pch + 1) * 128],
                        pt2[:].rearrange("p (a b) -> p a b", a=4))

            wf_cache = {}

            def load_wf(f):
                wf = ws.tile([128, DJ, 128], mt, name="wf", tag="wf")
                nc.sync.dma_start(wf[:], wfc_d[f])
                wf_cache[f] = wf

            def emit_fc1(c0, c1):
                for f in range(NF):
                    if f in wf_cache:
                        wf = wf_cache.pop(f)
                    else:
                        wf = ws.tile([128, DJ, 128], mt, name="wf", tag="wf")
                        nc.sync.dma_start(wf[:], wfc_d[f])
                    psf = pmm.tile([128, 512], f32, name="psf", tag="mm")
                    for j in range(DJ):
                        nc.tensor.matmul(
                            psf[:, 0:c1 - c0], wf[:, j, :],
                            h2T[:, j, c0:c1], start=(j == 0),
                            stop=(j == DJ - 1))
                    nc.vector.tensor_scalar(
                        m1T[:, f, c0:c1], psf[:, 0:c1 - c0],
                        bfc_sb[:, f:f + 1], 0.0, OP.add, OP.max)

            # ---------------- pipelined chunk loop ----------------
            # x for chunk 0 first so LN1 isn't stuck behind weight loads
            load_x(0)
            nc.sync.dma_start(Wq_sb[:], wq_d)
            nc.sync.dma_start(Wk_sb[:], wk_d)
            nc.sync.dma_start(Wv_sb[:], wv_d)
            nc.sync.dma_start(Wp_sb[:], wp_d)

            for ch in range(NCH):
                emit_ln1_qkv(ch)
                emit_attn(ch)
                emit_proj_rs(ch)
                if ch == 3:
                    for f in range(3):
                        load_wf(f)    # prefetch first fc1 weight tiles
                if ch >= 1:
                    emit_resid_ln2(ch - 1)
            emit_fc1(0, 384)      # chunks 0-2; overlaps the final RS
            emit_resid_ln2(3)
            emit_fc1(384, 512)

        # ---------------- fc2 + residual + output ----------------
        with tc.tile_pool(name="p6ps", bufs=1, space="PSUM") as p6ps, \
                tc.tile_pool(name="wos", bufs=2) as wos, \
                tc.tile_pool(name="p4o", bufs=2) as p4o:
            pso = [[p6ps.tile([128, 512], f32, name=f"pso_{tl}_{n}")
                    for n in range(2)] for tl in range(SLT)]
            for tl in range(SLT):
                for n in range(2):
                    nc.tensor.matmul(
                        pso[tl][n][:], ones1[:],
                        bout_sb[:, n * 512:(n + 1) * 512],
                        start=True, stop=False)
            for f in range(NF):
                wo = wos.tile([128, D], mt, name="wo", tag="wo")
                nc.sync.dma_start(wo[:], wo_d[f * 128:(f + 1) * 128, :])
                for tl in range(SLT):
                    for n in range(2):
                        nc.tensor.matmul(
                            pso[tl][n][:],
                            m1T[:, f, tl * 128:(tl + 1) * 128],
                            wo[:, n * 512:(n + 1) * 512],
                            start=False, stop=(f == NF - 1))
            for tl in range(SLT):
                for n in range(2):
                    ot = p4o.tile([128, 512], f32, name="ot", tag="ot")
                    nc.vector.tensor_tensor(
                        ot[:], pso[tl][n][:], xP[:, tl, n, :], OP.add)
                    nc.sync.dma_start(
                        out_d[tl * 128:(tl + 1) * 128,
                              n * 512:(n + 1) * 512], ot[:])
    nc.compile()
    return nc


def own_token_idx(t, S=S_FULL, GROUP=GROUP_FULL):
    CSG = CS // GROUP
    return np.concatenate([
        np.arange(qc * CS + t * CSG, qc * CS + (t + 1) * CSG)
        for qc in range(S // CS)])


def marshal_inputs(x, ln1_g, ln1_b, ln2_g, ln2_b, W_qkv, b_qkv, W_proj,
                   b_proj, W_fc, b_fc, W_out, b_out,
                   S=S_FULL, DFF=DFF_FULL, GROUP=GROUP_FULL,
                   n_cores=N_CORES):
    NF = DFF // 128
    import ml_dtypes
    adt = ml_dtypes.bfloat16
    mdt = ml_dtypes.bfloat16

    def f32c(a):
        return np.ascontiguousarray(a, dtype=np.float32)

    def ac(a):
        return np.ascontiguousarray(a, dtype=adt)

    def mc(a):
        return np.ascontiguousarray(a, dtype=mdt)

    # fold LN1 gamma/beta into W_qkv / b_qkv
    W_qkv_f = W_qkv * ln1_g[:, None]
    b_qkv_f = ln1_b @ W_qkv + b_qkv
    # fold LN2 gamma/beta into W_fc / b_fc
    W_fc_f = W_fc * ln2_g[:, None]
    b_fc_f = ln2_b @ W_fc + b_fc
    # fold the V bias through proj (softmax rows sum to 1), /4 because the
    # ReduceScatter sums 4 per-core partials each carrying bias/4
    bv_full = b_qkv_f[2 * D:3 * D]
    bprojq = (b_proj + bv_full @ W_proj) * 0.25

    base = {
        "bfc_m": f32c(b_fc_f.reshape(NF, 128).T),
        "wfc_m": mc(W_fc_f.reshape(DJ, 128, NF, 128).transpose(2, 1, 0, 3)),
        "wout_m": mc(W_out),
        "bprojq_m": f32c(bprojq.reshape(1, D)),
        "bout_m": mc(b_out.reshape(1, D)),
    }
    in_maps = []
    for c in range(n_cores):
        g, t = c // GROUP, c % GROUP
        cs, ce = t * 256, (t + 1) * 256
        wq = W_qkv_f[:, cs:ce]
        wk = W_qkv_f[:, D + cs:D + ce]
        wv = W_qkv_f[:, 2 * D + cs:2 * D + ce]
        bq = b_qkv_f[cs:ce]
        bk = b_qkv_f[D + cs:D + ce]
        wp = W_proj[cs:ce, :]
        m = dict(base)
        m["x_b"] = f32c(x[g])
        m["x_own"] = f32c(x[g][own_token_idx(t, S, GROUP)])
        m["wq_m"] = ac(wq.reshape(DJ, 128, 256).transpose(1, 0, 2))
        m["wk_m"] = ac(wk.reshape(DJ, 128, 256).transpose(1, 0, 2))
        m["wv_m"] = ac(wv.reshape(DJ, 128, 256).transpose(1, 0, 2))
        m["bq_m"] = f32c(bq.reshape(2, 128).T)
        m["bk_m"] = f32c(bk.reshape(2, 128).T)
        m["wproj_m"] = ac(
            wp.reshape(2, 2, 64, D).transpose(1, 2, 0, 3).reshape(128, 2, D))
        in_maps.append(m)
    return in_maps


_NC_CACHE = {}


def _get_nc():
    if "nc" not in _NC_CACHE:
        _NC_CACHE["nc"] = build_nc()
    return _NC_CACHE["nc"]


def kernel(**inputs):
    inputs = {k: np.asarray(v, dtype=np.float32) for k, v in inputs.items()}
    nc = _get_nc()
    in_maps = marshal_inputs(**inputs)
    r = run_bass_kernel_spmd(nc, in_maps, core_ids=list(range(N_CORES)))
    out = np.empty((B, S_FULL, D), np.float32)
    for c in range(N_CORES):
        g, t = c // GROUP_FULL, c % GROUP_FULL
        out[g, own_token_idx(t), :] = r.results[c]["out_s"]
    return out


# revision 26
# speedup vs baseline: 1.1204x; 1.1204x over previous
"""Trainium2 Bass kernel for a dense transformer block (B=2, S=2048, D=1024,
H=16, d_ff=4096), sharded over 8 NeuronCores.

Sharding: DP(2 groups over batch) x TP(4 cores over heads) for
LN1/QKV/attention/proj, pipelined per 512-token chunk with a per-chunk
bf16 ReduceScatter of the proj partials; then token-parallel MLP (each
core: 512 tokens, full MLP weights). Host assembles the 8 per-core
outputs.

v1 optimizations over the original baseline:
- LN gamma/beta folded into W_qkv / W_fc on the host; V-projection bias
  folded into the proj bias (softmax rows sum to 1), proj bias folded
  into the po copy as bias/4 (RS sums 4 partials).
- LayerNorm stats via bn_stats/bn_aggr (one DVE pass), rstd via
  sqrt + reciprocal_approx_fast (no DVE table reloads).
- Softmax normalization: reciprocal_approx_fast on the ones-column
  denominator + gpsimd partition_broadcast + single multiply (no psum
  copy, no bias add).
- ReduceScatter payload in bf16 (half the bytes).
- Residual + LN2 + h2T transpose pipelined per chunk (runs during the
  next chunk's attention); first half of fc1 runs under the last RS.
- LN transposes (xc^T @ diag(rstd)) in bf16 (1 cyc/row vs 4 for f32).
"""

from contextlib import ExitStack

import numpy as np

import concourse.bacc as bacc
import concourse.mybir as mybir
import concourse.tile as tile
from concourse.bass_utils import run_bass_kernel_spmd
from concourse.masks import make_identity

f32 = mybir.dt.float32
bf16 = mybir.dt.bfloat16
AF = mybir.ActivationFunctionType
OP = mybir.AluOpType

B = 2
S_FULL = 2048
D = 1024
H = 16
HD = 64
DFF_FULL = 4096
LN_EPS = 1e-5
N_CORES = 8
GROUP_FULL = 4
HPC = 4
DJ = D // 128
CS = 512


def build_nc(S=S_FULL, DFF=DFF_FULL, GROUP=GROUP_FULL, n_cores=N_CORES):
    at = bf16
    mt = bf16
    NCH = S // CS
    SL = S // GROUP
    SLT = SL // 128
    NF = DFF // 128
    KT = S // 128
    groups = [list(range(g * GROUP, (g + 1) * GROUP))
              for g in range(n_cores // GROUP)]

    nc = bacc.Bacc("TRN2", target_bir_lowering=False, debug=False,
                   num_devices=n_cores)

    def din(name, shape, dt=f32):
        return nc.dram_tensor(name, shape, dt, kind="ExternalInput").ap()

    x_d = din("x_b", [S, D])
    xo_d = din("x_own", [SL, D])
    wq_d = din("wq_m", [128, DJ, 256], at)
    wk_d = din("wk_m", [128, DJ, 256], at)
    wv_d = din("wv_m", [128, DJ, 256], at)
    bq_d = din("bq_m", [128, 2])
    bk_d = din("bk_m", [128, 2])
    wp_d = din("wproj_m", [128, 2, D], at)
    bpq_d = din("bprojq_m", [1, D])
    wfc_d = din("wfc_m", [NF, 128, DJ, 128], mt)
    bfc_d = din("bfc_m", [128, NF])
    wo_d = din("wout_m", [DFF, D], mt)
    bo_d = din("bout_m", [1, D], mt)
    out_d = nc.dram_tensor("out_s", [SL, D], f32, kind="ExternalOutput").ap()

    with tile.TileContext(nc) as tc, ExitStack() as st0:
        su = st0.enter_context(tc.tile_pool(name="setup", bufs=1))
        ws = st0.enter_context(tc.tile_pool(name="wstream", bufs=3))
        drp = st0.enter_context(tc.tile_pool(name="dram", bufs=1, space="DRAM"))

        cc_ins = [drp.tile([CS, D], at, name=f"cc_in{i}")
                  for i in range(NCH)]
        cc_outs = [drp.tile([128, D], at, name=f"cc_out{i}")
                   for i in range(NCH)]
        wu_in = drp.tile([4, 4], f32, name="wu_in")
        wu_out = drp.tile([16, 4], f32, name="wu_out")

        # warmup collective: syncs the cores before the pipeline so the
        # first real ReduceScatter doesn't absorb core-start skew
        with tc.tile_pool(name="wup", bufs=1) as wup:
            wt = wup.tile([4, 4], f32, name="wt")
            nc.vector.memset(wt[:], 1.0)
            nc.sync.dma_start(wu_in[:], wt[:])
            nc.gpsimd.collective_compute(
                "AllGather", OP.bypass, replica_groups=groups,
                ins=[wu_in[:].opt()], outs=[wu_out[:].opt()])

        ident = su.tile([128, 128], f32, name="ident")
        make_identity(nc, ident[:])
        negC = su.tile([128, 1], f32, name="negC")
        nc.vector.memset(negC[:], -4.0)
        epsb = su.tile([128, 1], f32, name="epsb")
        nc.vector.memset(epsb[:], LN_EPS)

        bq_sb = su.tile([128, 2], f32, name="bq_sb")
        nc.sync.dma_start(bq_sb[:], bq_d)
        bk_sb = su.tile([128, 2], f32, name="bk_sb")
        nc.sync.dma_start(bk_sb[:], bk_d)
        bfc_sb = su.tile([128, NF], f32, name="bfc_sb")
        nc.sync.dma_start(bfc_sb[:], bfc_d)
        bout_sb = su.tile([1, D], mt, name="bout_sb")
        nc.sync.dma_start(bout_sb[:], bo_d)
        ones1 = su.tile([1, 128], mt, name="ones1")
        nc.vector.memset(ones1[:], 1.0)

        bprojq_bc = su.tile([128, D], f32, name="bprojq_bc")
        with tc.tile_pool(name="tmpb", bufs=1) as tb:
            brow = tb.tile([1, D], f32, name="brow")
            nc.sync.dma_start(brow[:], bpq_d)
            nc.gpsimd.partition_broadcast(bprojq_bc[:], brow[:])

        # static causal masks for the diagonal key tiles: cmask[i][p, c] =
        # 1 if c - p >= i*128 else 0  (key-tile offset d0 = k0-q0 = i*128)
        cmask = su.tile([128, 4, 512], bf16, name="cmask")
        nc.vector.memset(cmask[:], 1.0)
        for i in range(4):
            nc.gpsimd.affine_select(
                out=cmask[:, i, :], in_=cmask[:, i, :],
                compare_op=OP.is_ge, fill=0.0, base=-i * 128,
                pattern=[[1, 512]], channel_multiplier=-1)

        # persistent attention-side tensors
        ap = st0.enter_context(tc.tile_pool(name="attn_per", bufs=1))
        Qt = ap.tile([128, 2, S], at, name="Qt")
        Kt = ap.tile([128, 2, S], at, name="Kt")
        yT = ap.tile([128, 2, S], at, name="yT")
        Vg = ap.tile([128, KT, HPC, 65], at, name="Vg")
        nc.vector.memset(Vg[:, :, :, 64:65], 1.0)
        Wp_sb = ap.tile([128, 2, D], at, name="Wp_sb")
        Wq_sb = ap.tile([128, DJ, 256], at, name="Wq_sb")
        Wk_sb = ap.tile([128, DJ, 256], at, name="Wk_sb")
        Wv_sb = ap.tile([128, DJ, 256], at, name="Wv_sb")

        # persistent MLP-side tensors
        xP = ap.tile([128, SLT, 2, 512], f32, name="xP")
        h2T = ap.tile([128, DJ, SL], mt, name="h2T")
        m1T = ap.tile([128, NF, SL], mt, name="m1T")

        with ExitStack() as st1:
            p1x = st1.enter_context(tc.tile_pool(name="p1x", bufs=4))
            p1xc = st1.enter_context(tc.tile_pool(name="p1xc", bufs=4))
            p1ht = st1.enter_context(tc.tile_pool(name="p1ht", bufs=2))
            p1d = st1.enter_context(tc.tile_pool(name="p1d", bufs=8))
            p1s = st1.enter_context(tc.tile_pool(name="p1s", bufs=8))
            p2e = st1.enter_context(tc.tile_pool(name="p2e", bufs=2))
            p2n = st1.enter_context(tc.tile_pool(name="p2n", bufs=2))
            p2o = st1.enter_context(tc.tile_pool(name="p2o", bufs=2))
            p4z = st1.enter_context(tc.tile_pool(name="p4z", bufs=2))
            p4xc = st1.enter_context(tc.tile_pool(name="p4xc", bufs=2))
            p4s = st1.enter_context(tc.tile_pool(name="p4s", bufs=8))
            pmm = st1.enter_context(tc.tile_pool(name="pmm", bufs=2,
                                                 space="PSUM"))
            pss = st1.enter_context(tc.tile_pool(name="pss", bufs=2,
                                                 space="PSUM"))
            p2y = st1.enter_context(tc.tile_pool(name="p2y", bufs=2,
                                                 space="PSUM"))

            x_cache = {}

            def load_x(ch):
                tiles = []
                for tl in range(4):
                    ti = ch * 4 + tl
                    xt = p1x.tile([128, 2, 512], f32, name="xt", tag="xt")
                    nc.sync.dma_start(
                        xt[:, 0, :], x_d[ti * 128:(ti + 1) * 128, 0:512])
                    nc.sync.dma_start(
                        xt[:, 1, :], x_d[ti * 128:(ti + 1) * 128, 512:1024])
                    tiles.append(xt)
                x_cache[ch] = tiles

            def emit_ln1_qkv(ch):
                # LayerNorm1 stats + centered x + diag(rstd), per token tile
                if ch not in x_cache:
                    load_x(ch)
                xts = x_cache.pop(ch)
                xcs, diags = [], []
                for tl in range(4):
                    xt = xts[tl]
                    bns = p1s.tile([128, 2, 6], f32, name="bns", tag="bns")
                    nc.vector.bn_stats(bns[:, 0, :], xt[:, 0, :])
                    nc.vector.bn_stats(bns[:, 1, :], xt[:, 1, :])
                    mv = p1s.tile([128, 2], f32, name="mv", tag="mv")
                    nc.vector.bn_aggr(mv[:], bns[:])
                    xc = p1xc.tile([128, 2, 512], at, name="xc", tag="xc")
                    nc.vector.tensor_scalar(
                        xc[:], xt[:], mv[:, 0:1], None, OP.subtract)
                    sd = p1s.tile([128, 1], f32, name="sd", tag="sd")
                    nc.scalar.activation(
                        sd[:], mv[:, 1:2], AF.Sqrt, bias=epsb[:], scale=1.0)
                    rstd = p1s.tile([128, 1], f32, name="rstd", tag="rstd")
                    nc.vector.reciprocal_approx_fast(rstd[:], sd[:])
                    dg = p1d.tile([128, 128], at, name="dg", tag="dg")
                    nc.vector.tensor_scalar_mul(dg[:], ident[:], rstd[:])
                    xcs.append(xc)
                    diags.append(dg)

                # h^T (normalized-x transpose) via diag matmuls; gamma/beta
                # already folded into W_qkv on the host.
                hT = p1ht.tile([128, DJ, CS], at, name="hT", tag="hT")
                for j in range(DJ):
                    ptt = pmm.tile([128, 512], f32, name="ptt", tag="mm")
                    for tl in range(4):
                        nc.tensor.matmul(
                            ptt[:, tl * 128:(tl + 1) * 128],
                            xcs[tl][:, j // 4, (j % 4) * 128:(j % 4) * 128 + 128],
                            diags[tl][:], start=True, stop=True)
                    nc.vector.tensor_copy(hT[:, j, :], ptt[:])

                # QKV
                for hp in range(2):
                    psq = pmm.tile([128, 512], f32, name="psq", tag="mm")
                    for j in range(DJ):
                        nc.tensor.matmul(
                            psq[:], Wq_sb[:, j, hp * 128:(hp + 1) * 128],
                            hT[:, j, :], start=(j == 0), stop=(j == DJ - 1))
                    nc.vector.tensor_scalar(
                        Qt[:, hp, ch * CS:(ch + 1) * CS], psq[:],
                        bq_sb[:, hp:hp + 1], None, OP.add)
                    psk = pmm.tile([128, 512], f32, name="psk", tag="mm")
                    for j in range(DJ):
                        nc.tensor.matmul(
                            psk[:], Wk_sb[:, j, hp * 128:(hp + 1) * 128],
                            hT[:, j, :], start=(j == 0), stop=(j == DJ - 1))
                    nc.vector.tensor_scalar(
                        Kt[:, hp, ch * CS:(ch + 1) * CS], psk[:],
                        bk_sb[:, hp:hp + 1], None, OP.add)
                for tl in range(4):
                    ti = ch * 4 + tl
                    psv = pmm.tile([128, 512], f32, name="psv", tag="mm")
                    for j in range(DJ):
                        nc.tensor.matmul(
                            psv[:, 0:256],
                            hT[:, j, tl * 128:(tl + 1) * 128],
                            Wv_sb[:, j, :], start=(j == 0),
                            stop=(j == DJ - 1))
                    for h in range(HPC):
                        nc.vector.tensor_copy(
                            Vg[:, ti, h, 0:64],
                            psv[:, h * 64:(h + 1) * 64])

            def emit_attn(ch):
                q0 = ch * CS
                nkj = (q0 + CS) // 128
                pending_norm = [None]
                # h2=1 heads first: their yT writes bounce through an
                # SBUF DMA, so get them off the proj critical path
                for hp in range(2):
                    for h2 in (1, 0):
                        h = hp * 2 + h2
                        psy = p2y.tile([128, CS], f32, name="psy", tag="psy")
                        first = True
                        for g0 in range(0, nkj, 2):
                            psc = pss.tile([128, 1024], f32, name="psc",
                                           tag="pss")
                            for kk in range(2):
                                kjt = g0 + kk
                                nc.tensor.matmul(
                                    psc[:, kk * 512:(kk + 1) * 512],
                                    Kt[h2 * 64:(h2 + 1) * 64, hp,
                                       kjt * 128:(kjt + 1) * 128],
                                    Qt[h2 * 64:(h2 + 1) * 64, hp,
                                       q0:q0 + CS],
                                    start=True, stop=True)
                            es = p2e.tile([128, 1024], at, name="es",
                                          tag="es")
                            nc.scalar.activation(
                                es[:], psc[:], AF.Exp, bias=negC[:],
                                scale=0.125)
                            for kk in range(2):
                                kjt = g0 + kk
                                k0 = kjt * 128
                                if k0 >= q0:
                                    nc.vector.tensor_tensor(
                                        es[:, kk * 512:(kk + 1) * 512],
                                        es[:, kk * 512:(kk + 1) * 512],
                                        cmask[:, (k0 - q0) // 128, :],
                                        OP.mult)
                                nc.tensor.matmul(
                                    psy[0:65, :], Vg[:, kjt, h, :],
                                    es[:, kk * 512:(kk + 1) * 512],
                                    start=first, stop=(kjt == nkj - 1))
                                first = False
                        # normalize is deferred one head so its K=1
                        # broadcast matmul doesn't stall the PE between
                        # heads (emitted after the next head's AV)
                        def norm(hp=hp, h2=h2, psy=psy):
                            den = p2n.tile([1, 512], f32, name="den",
                                           tag="den")
                            nc.vector.tensor_copy(den[:], psy[64:65, :])
                            ivf = p2n.tile([1, 512], f32, name="ivf",
                                           tag="ivf")
                            nc.vector.reciprocal_approx_fast(ivf[:], den[:])
                            ivh = p2n.tile([1, 512], at, name="ivh",
                                           tag="ivh")
                            nc.vector.tensor_copy(ivh[:], ivf[:])
                            psb = pmm.tile([128, 512], f32, name="psb",
                                           tag="mm")
                            nc.tensor.matmul(psb[0:64, :], ones1[:, 0:64],
                                             ivh[:], start=True, stop=True)
                            bcst = p2n.tile([64, 512], f32, name="bcst",
                                            tag="bcst")
                            nc.vector.tensor_copy(bcst[:], psb[0:64, :])
                            if h2 == 0:
                                nc.vector.tensor_tensor(
                                    yT[0:64, hp, q0:q0 + CS], psy[0:64, :],
                                    bcst[:], OP.mult)
                            else:
                                st2 = p2n.tile([64, 512], at, name="st2",
                                               tag="st2")
                                nc.vector.tensor_tensor(
                                    st2[:], psy[0:64, :], bcst[:], OP.mult)
                                nc.sync.dma_start(
                                    yT[64:128, hp, q0:q0 + CS], st2[:])
                        if pending_norm[0] is not None:
                            pending_norm[0]()
                        pending_norm[0] = norm
                pending_norm[0]()

            def emit_proj_rs(ch):
                for tl in range(4):
                    ti = ch * 4 + tl
                    for n in range(2):
                        psp = pmm.tile([128, 512], f32, name="psp", tag="mm")
                        for hp in range(2):
                            nc.tensor.matmul(
                                psp[:],
                                yT[:, hp, ti * 128:(ti + 1) * 128],
                                Wp_sb[:, hp, n * 512:(n + 1) * 512],
                                start=(hp == 0), stop=(hp == 1))
                        po = p2o.tile([128, 512], at, name="po", tag="po")
                        nc.vector.tensor_tensor(
                            po[:], psp[:],
                            bprojq_bc[:, n * 512:(n + 1) * 512], OP.add)
                        nc.sync.dma_start(
                            cc_ins[ch][tl * 128:(tl + 1) * 128,
                                       n * 512:(n + 1) * 512], po[:])
                nc.gpsimd.collective_compute(
                    "ReduceScatter", OP.add, replica_groups=groups,
                    ins=[cc_ins[ch][:].opt()],
                    outs=[cc_outs[ch][:].opt()])

            def emit_resid_ln2(pch):
                zt = p4z.tile([128, 2, 512], at, name="zt", tag="zt")
                nc.sync.dma_start(zt[:, 0, :], cc_outs[pch][:, 0:512])
                nc.sync.dma_start(zt[:, 1, :], cc_outs[pch][:, 512:1024])
                xre = p4z.tile([128, 2, 512], f32, name="xre", tag="xre")
                nc.sync.dma_start(
                    xre[:, 0, :], xo_d[pch * 128:(pch + 1) * 128, 0:512])
                nc.sync.dma_start(
                    xre[:, 1, :], xo_d[pch * 128:(pch + 1) * 128, 512:1024])
                xPc = xP[:, pch, :, :]
                nc.vector.tensor_tensor(xPc, zt[:], xre[:], OP.add)
                bns2 = p4s.tile([128, 2, 6], f32, name="bns2", tag="bns2")
                nc.vector.bn_stats(bns2[:, 0, :], xPc[:, 0, :])
                nc.vector.bn_stats(bns2[:, 1, :], xPc[:, 1, :])
                mv2 = p4s.tile([128, 2], f32, name="mv2", tag="mv2")
                nc.vector.bn_aggr(mv2[:], bns2[:])
                xc2 = p4xc.tile([128, 2, 512], at, name="xc2", tag="xc2")
                nc.vector.tensor_scalar(
                    xc2[:], xPc, mv2[:, 0:1], None, OP.subtract)
                sd2 = p4s.tile([128, 1], f32, name="sd2", tag="sd2")
                nc.scalar.activation(
                    sd2[:], mv2[:, 1:2], AF.Sqrt, bias=epsb[:], scale=1.0)
                rstd2 = p4s.tile([128, 1], f32, name="rstd2", tag="rstd2")
                nc.vector.reciprocal_approx_fast(rstd2[:], sd2[:])
                dg2 = p1d.tile([128, 128], at, name="dg2", tag="dg")
                nc.vector.tensor_scalar_mul(dg2[:], ident[:], rstd2[:])
                for jg in range(2):
                    pt2 = pmm.tile([128, 512], f32, name="pt2", tag="mm")
                    for jj in range(4):
                        j = jg * 4 + jj
                        nc.tensor.matmul(
                            pt2[:, jj * 128:(jj + 1) * 128],
                            xc2[:, j // 4, (j % 4) * 128:(j % 4) * 128 + 128],
                            dg2[:], start=True, stop=True)
                    nc.vector.tensor_copy(
                        h2T[:, jg * 4:(jg + 1) * 4,
                            pch * 128:(pch + 1) * 128],
                        pt2[:].rearrange("p (a b) -> p a b", a=4))

            wf_cache = {}

            def load_wf(f):
                wf = ws.tile([128, DJ, 128], mt, name="wf", tag="wf")
                nc.sync.dma_start(wf[:], wfc_d[f])
                wf_cache[f] = wf

            def emit_fc1(c0, c1):
                for f in range(NF):
                    if f in wf_cache:
                        wf = wf_cache.pop(f)
                    else:
                        wf = ws.tile([128, DJ, 128], mt, name="wf", tag="wf")
                        nc.sync.dma_start(wf[:], wfc_d[f])
                    psf = pmm.tile([128, 512], f32, name="psf", tag="mm")
                    for j in range(DJ):
                        nc.tensor.matmul(
                            psf[:, 0:c1 - c0], wf[:, j, :],
                            h2T[:, j, c0:c1], start=(j == 0),
                            stop=(j == DJ - 1))
                    nc.vector.tensor_scalar(
                        m1T[:, f, c0:c1], psf[:, 0:c1 - c0],
                        bfc_sb[:, f:f + 1], 0.0, OP.add, OP.max)

            # ---------------- pipelined chunk loop ----------------
            # x for chunk 0 first so LN1 isn't stuck behind weight loads
            load_x(0)
            nc.sync.dma_start(Wq_sb[:], wq_d)
            nc.sync.dma_start(Wk_sb[:], wk_d)
            nc.sync.dma_start(Wv_sb[:], wv_d)
            nc.sync.dma_start(Wp_sb[:], wp_d)

            for ch in range(NCH):
                emit_ln1_qkv(ch)
                emit_attn(ch)
                emit_proj_rs(ch)
                if ch == 3:
                    for f in range(3):
                        load_wf(f)    # prefetch first fc1 weight tiles
                if ch >= 1:
                    emit_resid_ln2(ch - 1)
            emit_fc1(0, 384)      # chunks 0-2; overlaps the final RS
            emit_resid_ln2(3)
            emit_fc1(384, 512)

        # ---------------- fc2 + residual + output ----------------
        with tc.tile_pool(name="p6ps", bufs=1, space="PSUM") as p6ps, \
                tc.tile_pool(name="wos", bufs=2) as wos, \
                tc.tile_pool(name="p4o", bufs=2) as p4o:
            pso = [[p6ps.tile([128, 512], f32, name=f"pso_{tl}_{n}")
                    for n in range(2)] for tl in range(SLT)]
            for tl in range(SLT):
                for n in range(2):
                    nc.tensor.matmul(
                        pso[tl][n][:], ones1[:],
                        bout_sb[:, n * 512:(n + 1) * 512],
                        start=True, stop=False)
            for f in range(NF):
                wo = wos.tile([128, D], mt, name="wo", tag="wo")
                nc.sync.dma_start(wo[:], wo_d[f * 128:(f + 1) * 128, :])
                for tl in range(SLT):
                    for n in range(2):
                        nc.tensor.matmul(
                            pso[tl][n][:],
                            m1T[:, f, tl * 128:(tl + 1) * 128],
                            wo[:, n * 512:(n + 1) * 512],
                            start=False, stop=(f == NF - 1))
            for tl in range(SLT):
                for n in range(2):
                    ot = p4o.tile([128, 512], f32, name="ot", tag="ot")
                    nc.vector.tensor_tensor(
                        ot[:], pso[tl][n][:], xP[:, tl, n, :], OP.add)
                    nc.sync.dma_start(
                        out_d[tl * 128:(tl + 1) * 128,
                              n * 512:(n + 1) * 512], ot[:])
    nc.compile()
    return nc


def own_token_idx(t, S=S_FULL, GROUP=GROUP_FULL):
    CSG = CS // GROUP
    return np.concatenate([
        np.arange(qc * CS + t * CSG, qc * CS + (t + 1) * CSG)
        for qc in range(S // CS)])


def marshal_inputs(x, ln1_g, ln1_b, ln2_g, ln2_b, W_qkv, b_qkv, W_proj,
                   b_proj, W_fc, b_fc, W_out, b_out,
                   S=S_FULL, DFF=DFF_FULL, GROUP=GROUP_FULL,
                   n_cores=N_CORES):
    NF = DFF // 128
    import ml_dtypes
    adt = ml_dtypes.bfloat16
    mdt = ml_dtypes.bfloat16

    def f32c(a):
        return np.ascontiguousarray(a, dtype=np.float32)

    def ac(a):
        return np.ascontiguousarray(a, dtype=adt)

    def mc(a):
        return np.ascontiguousarray(a, dtype=mdt)

    # fold LN1 gamma/beta into W_qkv / b_qkv
    W_qkv_f = W_qkv * ln1_g[:, None]
    b_qkv_f = ln1_b @ W_qkv + b_qkv
    # fold LN2 gamma/beta into W_fc / b_fc
    W_fc_f = W_fc * ln2_g[:, None]
    b_fc_f = ln2_b @ W_fc + b_fc
    # fold the V bias through proj (softmax rows sum to 1), /4 because the
    # ReduceScatter sums 4 per-core partials each carrying bias/4
    bv_full = b_qkv_f[2 * D:3 * D]
    bprojq = (b_proj + bv_full @ W_proj) * 0.25

    base = {
        "bfc_m": f32c(b_fc_f.reshape(NF, 128).T),
        "wfc_m": mc(W_fc_f.reshape(DJ, 128, NF, 128).transpose(2, 1, 0, 3)),
        "wout_m": mc(W_out),
        "bprojq_m": f32c(bprojq.reshape(1, D)),
        "bout_m": mc(b_out.reshape(1, D)),
    }
    in_maps = []
    for c in range(n_cores):
        g, t = c // GROUP, c % GROUP
        cs, ce = t * 256, (t + 1) * 256
        wq = W_qkv_f[:, cs:ce]
        wk = W_qkv_f[:, D + cs:D + ce]
        wv = W_qkv_f[:, 2 * D + cs:2 * D + ce]
        bq = b_qkv_f[cs:ce]
        bk = b_qkv_f[D + cs:D + ce]
        wp = W_proj[cs:ce, :]
        m = dict(base)
        m["x_b"] = f32c(x[g])
        m["x_own"] = f32c(x[g][own_token_idx(t, S, GROUP)])
        m["wq_m"] = ac(wq.reshape(DJ, 128, 256).transpose(1, 0, 2))
        m["wk_m"] = ac(wk.reshape(DJ, 128, 256).transpose(1, 0, 2))
        m["wv_m"] = ac(wv.reshape(DJ, 128, 256).transpose(1, 0, 2))
        m["bq_m"] = f32c(bq.reshape(2, 128).T)
        m["bk_m"] = f32c(bk.reshape(2, 128).T)
        m["wproj_m"] = ac(
            wp.reshape(2, 2, 64, D).transpose(1, 2, 0, 3).reshape(128, 2, D))
        in_maps.append(m)
    return in_maps


_NC_CACHE = {}


def _get_nc():
    if "nc" not in _NC_CACHE:
        _NC_CACHE["nc"] = build_nc()
    return _NC_CACHE["nc"]


def kernel(**inputs):
    inputs = {k: np.asarray(v, dtype=np.float32) for k, v in inputs.items()}
    nc = _get_nc()
    in_maps = marshal_inputs(**inputs)
    r = run_bass_kernel_spmd(nc, in_maps, core_ids=list(range(N_CORES)))
    out = np.empty((B, S_FULL, D), np.float32)
    for c in range(N_CORES):
        g, t = c // GROUP_FULL, c % GROUP_FULL
        out[g, own_token_idx(t), :] = r.results[c]["out_s"]
    return out


# revision 32
# speedup vs baseline: 1.1404x; 1.0178x over previous
"""Trainium2 Bass kernel for a dense transformer block (B=2, S=2048, D=1024,
H=16, d_ff=4096), sharded over 8 NeuronCores.

Sharding: DP(2 groups over batch) x TP(4 cores over heads) for
LN1/QKV/attention/proj, pipelined per 512-token chunk with a per-chunk
bf16 ReduceScatter of the proj partials; then token-parallel MLP (each
core: 512 tokens, full MLP weights). Host assembles the 8 per-core
outputs.

v1 optimizations over the original baseline:
- LN gamma/beta folded into W_qkv / W_fc on the host; V-projection bias
  folded into the proj bias (softmax rows sum to 1), proj bias folded
  into the po copy as bias/4 (RS sums 4 partials).
- LayerNorm stats via bn_stats/bn_aggr (one DVE pass), rstd via
  sqrt + reciprocal_approx_fast (no DVE table reloads).
- Softmax normalization: reciprocal_approx_fast on the ones-column
  denominator + gpsimd partition_broadcast + single multiply (no psum
  copy, no bias add).
- ReduceScatter payload in bf16 (half the bytes).
- Residual + LN2 + h2T transpose pipelined per chunk (runs during the
  next chunk's attention); first half of fc1 runs under the last RS.
- LN transposes (xc^T @ diag(rstd)) in bf16 (1 cyc/row vs 4 for f32).
"""

from contextlib import ExitStack

import numpy as np

import concourse.bacc as bacc
import concourse.mybir as mybir
import concourse.tile as tile
from concourse.bass_utils import run_bass_kernel_spmd
from concourse.masks import make_identity

f32 = mybir.dt.float32
bf16 = mybir.dt.bfloat16
AF = mybir.ActivationFunctionType
OP = mybir.AluOpType

B = 2
S_FULL = 2048
D = 1024
H = 16
HD = 64
DFF_FULL = 4096
LN_EPS = 1e-5
N_CORES = 8
GROUP_FULL = 4
HPC = 4
DJ = D // 128
CS = 512


def build_nc(S=S_FULL, DFF=DFF_FULL, GROUP=GROUP_FULL, n_cores=N_CORES):
    at = bf16
    mt = bf16
    NCH = S // CS
    SL = S // GROUP
    SLT = SL // 128
    NF = DFF // 128
    KT = S // 128
    groups = [list(range(g * GROUP, (g + 1) * GROUP))
              for g in range(n_cores // GROUP)]

    nc = bacc.Bacc("TRN2", target_bir_lowering=False, debug=False,
                   num_devices=n_cores)

    def din(name, shape, dt=f32):
        return nc.dram_tensor(name, shape, dt, kind="ExternalInput").ap()

    x_d = din("x_b", [S, D])
    xo_d = din("x_own", [SL, D])
    wq_d = din("wq_m", [128, DJ, 256], at)
    wk_d = din("wk_m", [128, DJ, 256], at)
    wv_d = din("wv_m", [128, DJ, 256], at)
    bq_d = din("bq_m", [128, 2])
    bk_d = din("bk_m", [128, 2])
    wp_d = din("wproj_m", [128, 2, D], at)
    bpq_d = din("bprojq_m", [1, D])
    wfc_d = din("wfc_m", [NF, 128, DJ, 128], mt)
    bfc_d = din("bfc_m", [128, NF])
    wo_d = din("wout_m", [DFF, D], mt)
    bo_d = din("bout_m", [1, D], mt)
    out_d = nc.dram_tensor("out_s", [SL, D], f32, kind="ExternalOutput").ap()

    with tile.TileContext(nc) as tc, ExitStack() as st0:
        su = st0.enter_context(tc.tile_pool(name="setup", bufs=1))
        ws = st0.enter_context(tc.tile_pool(name="wstream", bufs=3))
        drp = st0.enter_context(tc.tile_pool(name="dram", bufs=1, space="DRAM"))

        cc_ins = [drp.tile([CS, D], at, name=f"cc_in{i}")
                  for i in range(NCH)]
        cc_outs = [drp.tile([128, D], at, name=f"cc_out{i}")
                   for i in range(NCH)]
        wu_in = drp.tile([4, 4], f32, name="wu_in")
        wu_out = drp.tile([16, 4], f32, name="wu_out")

        # warmup collective: syncs the cores before the pipeline so the
        # first real ReduceScatter doesn't absorb core-start skew
        with tc.tile_pool(name="wup", bufs=1) as wup:
            wt = wup.tile([4, 4], f32, name="wt")
            nc.vector.memset(wt[:], 1.0)
            nc.sync.dma_start(wu_in[:], wt[:])
            nc.gpsimd.collective_compute(
                "AllGather", OP.bypass, replica_groups=groups,
                ins=[wu_in[:].opt()], outs=[wu_out[:].opt()])

        ident = su.tile([128, 128], f32, name="ident")
        make_identity(nc, ident[:])
        negC = su.tile([128, 1], f32, name="negC")
        nc.vector.memset(negC[:], -4.0)
        epsb = su.tile([128, 1], f32, name="epsb")
        nc.vector.memset(epsb[:], LN_EPS)

        bq_sb = su.tile([128, 2], f32, name="bq_sb")
        nc.sync.dma_start(bq_sb[:], bq_d)
        bk_sb = su.tile([128, 2], f32, name="bk_sb")
        nc.sync.dma_start(bk_sb[:], bk_d)
        bfc_sb = su.tile([128, NF], f32, name="bfc_sb")
        nc.sync.dma_start(bfc_sb[:], bfc_d)
        bout_sb = su.tile([1, D], mt, name="bout_sb")
        nc.sync.dma_start(bout_sb[:], bo_d)
        ones1 = su.tile([1, 128], mt, name="ones1")
        nc.vector.memset(ones1[:], 1.0)

        bprojq_bc = su.tile([128, D], f32, name="bprojq_bc")
        with tc.tile_pool(name="tmpb", bufs=1) as tb:
            brow = tb.tile([1, D], f32, name="brow")
            nc.sync.dma_start(brow[:], bpq_d)
            nc.gpsimd.partition_broadcast(bprojq_bc[:], brow[:])

        # static causal masks for the diagonal key tiles: cmask[i][p, c] =
        # 1 if c - p >= i*128 else 0  (key-tile offset d0 = k0-q0 = i*128)
        cmask = su.tile([128, 4, 512], bf16, name="cmask")
        nc.vector.memset(cmask[:], 1.0)
        for i in range(4):
            nc.gpsimd.affine_select(
                out=cmask[:, i, :], in_=cmask[:, i, :],
                compare_op=OP.is_ge, fill=0.0, base=-i * 128,
                pattern=[[1, 512]], channel_multiplier=-1)

        # persistent attention-side tensors
        ap = st0.enter_context(tc.tile_pool(name="attn_per", bufs=1))
        Qt = ap.tile([128, 2, S], at, name="Qt")
        Kt = ap.tile([128, 2, S], at, name="Kt")
        yT = ap.tile([128, 2, S], at, name="yT")
        Vg = ap.tile([128, KT, HPC, 65], at, name="Vg")
        nc.vector.memset(Vg[:, :, :, 64:65], 1.0)
        Wp_sb = ap.tile([128, 2, D], at, name="Wp_sb")
        Wq_sb = ap.tile([128, DJ, 256], at, name="Wq_sb")
        Wk_sb = ap.tile([128, DJ, 256], at, name="Wk_sb")
        Wv_sb = ap.tile([128, DJ, 256], at, name="Wv_sb")

        # persistent MLP-side tensors
        xP = ap.tile([128, SLT, 2, 512], f32, name="xP")
        h2T = ap.tile([128, DJ, SL], mt, name="h2T")
        m1T = ap.tile([128, NF, SL], mt, name="m1T")

        with ExitStack() as st1:
            p1x = st1.enter_context(tc.tile_pool(name="p1x", bufs=3))
            p1xc = st1.enter_context(tc.tile_pool(name="p1xc", bufs=4))
            p1ht = st1.enter_context(tc.tile_pool(name="p1ht", bufs=2))
            p1d = st1.enter_context(tc.tile_pool(name="p1d", bufs=8))
            p1s = st1.enter_context(tc.tile_pool(name="p1s", bufs=8))
            p2e = st1.enter_context(tc.tile_pool(name="p2e", bufs=2))
            p2n = st1.enter_context(tc.tile_pool(name="p2n", bufs=2))
            p2o = st1.enter_context(tc.tile_pool(name="p2o", bufs=2))
            p4z = st1.enter_context(tc.tile_pool(name="p4z", bufs=1))
            p4xc = st1.enter_context(tc.tile_pool(name="p4xc", bufs=2))
            p4s = st1.enter_context(tc.tile_pool(name="p4s", bufs=8))
            wos = st1.enter_context(tc.tile_pool(name="wos", bufs=2))
            p4o = st1.enter_context(tc.tile_pool(name="p4o", bufs=2))
            pmm = st1.enter_context(tc.tile_pool(name="pmm", bufs=2,
                                                 space="PSUM"))
            # attention-only psum pools; closed before the MLP so fc2 can
            # hold 4 accumulator banks
            st_att = st1.enter_context(ExitStack())
            pss = st_att.enter_context(tc.tile_pool(name="pss", bufs=2,
                                                    space="PSUM"))
            p2y = st_att.enter_context(tc.tile_pool(name="p2y", bufs=2,
                                                    space="PSUM"))

            x_cache = {}

            def load_x(ch):
                tiles = []
                for tl in range(4):
                    ti = ch * 4 + tl
                    xt = p1x.tile([128, 2, 512], f32, name="xt", tag="xt")
                    nc.sync.dma_start(
                        xt[:, 0, :], x_d[ti * 128:(ti + 1) * 128, 0:512])
                    nc.sync.dma_start(
                        xt[:, 1, :], x_d[ti * 128:(ti + 1) * 128, 512:1024])
                    tiles.append(xt)
                x_cache[ch] = tiles

            def emit_ln1_qkv(ch):
                # LayerNorm1 stats + centered x + diag(rstd), per token tile
                if ch not in x_cache:
                    load_x(ch)
                xts = x_cache.pop(ch)
                xcs, diags = [], []
                for tl in range(4):
                    xt = xts[tl]
                    bns = p1s.tile([128, 2, 6], f32, name="bns", tag="bns")
                    nc.vector.bn_stats(bns[:, 0, :], xt[:, 0, :])
                    nc.vector.bn_stats(bns[:, 1, :], xt[:, 1, :])
                    mv = p1s.tile([128, 2], f32, name="mv", tag="mv")
                    nc.vector.bn_aggr(mv[:], bns[:])
                    xc = p1xc.tile([128, 2, 512], at, name="xc", tag="xc")
                    nc.vector.tensor_scalar(
                        xc[:], xt[:], mv[:, 0:1], None, OP.subtract)
                    sd = p1s.tile([128, 1], f32, name="sd", tag="sd")
                    nc.scalar.activation(
                        sd[:], mv[:, 1:2], AF.Sqrt, bias=epsb[:], scale=1.0)
                    rstd = p1s.tile([128, 1], f32, name="rstd", tag="rstd")
                    nc.vector.reciprocal_approx_fast(rstd[:], sd[:])
                    dg = p1d.tile([128, 128], at, name="dg", tag="dg")
                    nc.vector.tensor_scalar_mul(dg[:], ident[:], rstd[:])
                    xcs.append(xc)
                    diags.append(dg)

                # h^T (normalized-x transpose) via diag matmuls; gamma/beta
                # already folded into W_qkv on the host.
                hT = p1ht.tile([128, DJ, CS], at, name="hT", tag="hT")
                for j in range(DJ):
                    ptt = pmm.tile([128, 512], f32, name="ptt", tag="mm")
                    for tl in range(4):
                        nc.tensor.matmul(
                            ptt[:, tl * 128:(tl + 1) * 128],
                            xcs[tl][:, j // 4, (j % 4) * 128:(j % 4) * 128 + 128],
                            diags[tl][:], start=True, stop=True)
                    # psum->sbuf copy on the scalar engine (vector is the
                    # bottleneck in this phase, scalar has slack)
                    nc.scalar.activation(hT[:, j, :], ptt[:], AF.Copy)

                # QKV
                for hp in range(2):
                    psq = pmm.tile([128, 512], f32, name="psq", tag="mm")
                    for j in range(DJ):
                        nc.tensor.matmul(
                            psq[:], Wq_sb[:, j, hp * 128:(hp + 1) * 128],
                            hT[:, j, :], start=(j == 0), stop=(j == DJ - 1))
                    nc.vector.tensor_scalar(
                        Qt[:, hp, ch * CS:(ch + 1) * CS], psq[:],
                        bq_sb[:, hp:hp + 1], None, OP.add)
                    psk = pmm.tile([128, 512], f32, name="psk", tag="mm")
                    for j in range(DJ):
                        nc.tensor.matmul(
                            psk[:], Wk_sb[:, j, hp * 128:(hp + 1) * 128],
                            hT[:, j, :], start=(j == 0), stop=(j == DJ - 1))
                    nc.vector.tensor_scalar(
                        Kt[:, hp, ch * CS:(ch + 1) * CS], psk[:],
                        bk_sb[:, hp:hp + 1], None, OP.add)
                for tl in range(4):
                    ti = ch * 4 + tl
                    psv = pmm.tile([128, 512], f32, name="psv", tag="mm")
                    for j in range(DJ):
                        nc.tensor.matmul(
                            psv[:, 0:256],
                            hT[:, j, tl * 128:(tl + 1) * 128],
                            Wv_sb[:, j, :], start=(j == 0),
                            stop=(j == DJ - 1))
                    nc.scalar.activation(
                        Vg[:, ti, :, 0:64],
                        psv[:, 0:256].rearrange("p (a b) -> p a b", a=4),
                        AF.Copy)

            def emit_attn(ch):
                q0 = ch * CS
                nkj = (q0 + CS) // 128
                pending_norm = [None]
                # h2=1 heads first: their yT writes bounce through an
                # SBUF DMA, so get them off the proj critical path
                for hp in range(2):
                    for h2 in (1, 0):
                        h = hp * 2 + h2
                        psy = p2y.tile([128, CS], f32, name="psy", tag="psy")
                        first = True
                        for g0 in range(0, nkj, 2):
                            psc = pss.tile([128, 1024], f32, name="psc",
                                           tag="pss")
                            for kk in range(2):
                                kjt = g0 + kk
                                nc.tensor.matmul(
                                    psc[:, kk * 512:(kk + 1) * 512],
                                    Kt[h2 * 64:(h2 + 1) * 64, hp,
                                       kjt * 128:(kjt + 1) * 128],
                                    Qt[h2 * 64:(h2 + 1) * 64, hp,
                                       q0:q0 + CS],
                                    start=True, stop=True)
                            es = p2e.tile([128, 1024], at, name="es",
                                          tag="es")
                            nc.scalar.activation(
                                es[:], psc[:], AF.Exp, bias=negC[:],
                                scale=0.125)
                            if g0 * 128 >= q0:
                                # both tiles of this pair straddle the
                                # diagonal: one fused [128,1024] mask mult
                                i0 = (g0 * 128 - q0) // 128
                                nc.vector.tensor_tensor(
                                    es[:], es[:],
                                    cmask[:, i0:i0 + 2, :].rearrange(
                                        "p a b -> p (a b)"),
                                    OP.mult)
                            for kk in range(2):
                                kjt = g0 + kk
                                nc.tensor.matmul(
                                    psy[0:65, :], Vg[:, kjt, h, :],
                                    es[:, kk * 512:(kk + 1) * 512],
                                    start=first, stop=(kjt == nkj - 1))
                                first = False
                        # normalize is deferred one head so its K=1
                        # broadcast matmul doesn't stall the PE between
                        # heads (emitted after the next head's AV)
                        def norm(hp=hp, h2=h2, psy=psy):
                            den = p2n.tile([1, 512], f32, name="den",
                                           tag="den")
                            nc.vector.tensor_copy(den[:], psy[64:65, :])
                            ivf = p2n.tile([1, 512], f32, name="ivf",
                                           tag="ivf")
                            nc.vector.reciprocal_approx_fast(ivf[:], den[:])
                            ivh = p2n.tile([1, 512], at, name="ivh",
                                           tag="ivh")
                            nc.vector.tensor_copy(ivh[:], ivf[:])
                            psb = pmm.tile([128, 512], f32, name="psb",
                                           tag="mm")
                            nc.tensor.matmul(psb[0:64, :], ones1[:, 0:64],
                                             ivh[:], start=True, stop=True)
                            bcst = p2n.tile([64, 512], f32, name="bcst",
                                            tag="bcst")
                            nc.vector.tensor_copy(bcst[:], psb[0:64, :])
                            if h2 == 0:
                                nc.vector.tensor_tensor(
                                    yT[0:64, hp, q0:q0 + CS], psy[0:64, :],
                                    bcst[:], OP.mult)
                            else:
                                st2 = p2n.tile([64, 512], at, name="st2",
                                               tag="st2")
                                nc.vector.tensor_tensor(
                                    st2[:], psy[0:64, :], bcst[:], OP.mult)
                                nc.sync.dma_start(
                                    yT[64:128, hp, q0:q0 + CS], st2[:])
                        if pending_norm[0] is not None:
                            pending_norm[0]()
                        pending_norm[0] = norm
                pending_norm[0]()

            def emit_proj_rs(ch):
                for tl in range(4):
                    ti = ch * 4 + tl
                    for n in range(2):
                        psp = pmm.tile([128, 512], f32, name="psp", tag="mm")
                        for hp in range(2):
                            nc.tensor.matmul(
                                psp[:],
                                yT[:, hp, ti * 128:(ti + 1) * 128],
                                Wp_sb[:, hp, n * 512:(n + 1) * 512],
                                start=(hp == 0), stop=(hp == 1))
                        po = p2o.tile([128, 512], at, name="po", tag="po")
                        nc.vector.tensor_tensor(
                            po[:], psp[:],
                            bprojq_bc[:, n * 512:(n + 1) * 512], OP.add)
                        nc.sync.dma_start(
                            cc_ins[ch][tl * 128:(tl + 1) * 128,
                                       n * 512:(n + 1) * 512], po[:])
                nc.gpsimd.collective_compute(
                    "ReduceScatter", OP.add, replica_groups=groups,
                    ins=[cc_ins[ch][:].opt()],
                    outs=[cc_outs[ch][:].opt()])

            def emit_resid_ln2(pch):
                zt = p4z.tile([128, 2, 512], at, name="zt", tag="zt")
                nc.sync.dma_start(zt[:, 0, :], cc_outs[pch][:, 0:512])
                nc.sync.dma_start(zt[:, 1, :], cc_outs[pch][:, 512:1024])
                xre = p4z.tile([128, 2, 512], f32, name="xre", tag="xre")
                nc.sync.dma_start(
                    xre[:, 0, :], xo_d[pch * 128:(pch + 1) * 128, 0:512])
                nc.sync.dma_start(
                    xre[:, 1, :], xo_d[pch * 128:(pch + 1) * 128, 512:1024])
                xPc = xP[:, pch, :, :]
                nc.vector.tensor_tensor(xPc, zt[:], xre[:], OP.add)
                bns2 = p4s.tile([128, 2, 6], f32, name="bns2", tag="bns2")
                nc.vector.bn_stats(bns2[:, 0, :], xPc[:, 0, :])
                nc.vector.bn_stats(bns2[:, 1, :], xPc[:, 1, :])
                mv2 = p4s.tile([128, 2], f32, name="mv2", tag="mv2")
                nc.vector.bn_aggr(mv2[:], bns2[:])
                xc2 = p4xc.tile([128, 2, 512], at, name="xc2", tag="xc2")
                nc.vector.tensor_scalar(
                    xc2[:], xPc, mv2[:, 0:1], None, OP.subtract)
                sd2 = p4s.tile([128, 1], f32, name="sd2", tag="sd2")
                nc.scalar.activation(
                    sd2[:], mv2[:, 1:2], AF.Sqrt, bias=epsb[:], scale=1.0)
                rstd2 = p4s.tile([128, 1], f32, name="rstd2", tag="rstd2")
                nc.vector.reciprocal_approx_fast(rstd2[:], sd2[:])
                dg2 = p1d.tile([128, 128], at, name="dg2", tag="dg")
                nc.vector.tensor_scalar_mul(dg2[:], ident[:], rstd2[:])
                for jg in range(2):
                    pt2 = pmm.tile([128, 512], f32, name="pt2", tag="mm")
                    for jj in range(4):
                        j = jg * 4 + jj
                        nc.tensor.matmul(
                            pt2[:, jj * 128:(jj + 1) * 128],
                            xc2[:, j // 4, (j % 4) * 128:(j % 4) * 128 + 128],
                            dg2[:], start=True, stop=True)
                    nc.vector.tensor_copy(
                        h2T[:, jg * 4:(jg + 1) * 4,
                            pch * 128:(pch + 1) * 128],
                        pt2[:].rearrange("p (a b) -> p a b", a=4))

            wf_cache = {}

            def load_wf(f):
                wf = ws.tile([128, DJ, 128], mt, name="wf", tag="wf")
                nc.sync.dma_start(wf[:], wfc_d[f])
                wf_cache[f] = wf

            # ---------------- pipelined chunk loop ----------------
            # x for chunk 0 first so LN1 isn't stuck behind weight loads
            load_x(0)
            nc.sync.dma_start(Wq_sb[:], wq_d)
            nc.sync.dma_start(Wk_sb[:], wk_d)
            nc.sync.dma_start(Wv_sb[:], wv_d)
            nc.sync.dma_start(Wp_sb[:], wp_d)

            for ch in range(NCH):
                emit_ln1_qkv(ch)
                emit_attn(ch)
                emit_proj_rs(ch)
                if ch == 3:
                    for f in range(3):
                        load_wf(f)    # prefetch first fc1 weight tiles
                if ch >= 1:
                    emit_resid_ln2(ch - 1)
            # ---------------- MLP: fc1/fc2 interleaved ----------------
            # attention psum pools are closed; 4 fc2 accumulator banks fit
            st_att.close()
            p5ps = st1.enter_context(tc.tile_pool(name="p5ps", bufs=1,
                                                  space="PSUM"))

            def emit_fc1_f(f, c0, c1):
                if f in wf_cache:
                    wf = wf_cache.pop(f)
                else:
                    wf = ws.tile([128, DJ, 128], mt, name="wf", tag="wf")
                    nc.sync.dma_start(wf[:], wfc_d[f])
                psf = pmm.tile([128, 512], f32, name="psf", tag="mm")
                for j in range(DJ):
                    nc.tensor.matmul(
                        psf[:, 0:c1 - c0], wf[:, j, :],
                        h2T[:, j, c0:c1], start=(j == 0),
                        stop=(j == DJ - 1))
                nc.vector.tensor_scalar(
                    m1T[:, f, c0:c1], psf[:, 0:c1 - c0],
                    bfc_sb[:, f:f + 1], 0.0, OP.add, OP.max)

            def fc2_alloc_init(tls):
                pso = [p5ps.tile([128, 512], f32, name=f"pso{i}",
                                 tag=f"pso{i}") for i in range(4)]
                for i, t in enumerate(pso):
                    n = i % 2
                    nc.tensor.matmul(
                        t[:], ones1[:], bout_sb[:, n * 512:(n + 1) * 512],
                        start=True, stop=False)
                return pso

            def emit_fc2_f(pso, f, tls):
                wo = wos.tile([128, D], mt, name="wo", tag="wo")
                nc.sync.dma_start(wo[:], wo_d[f * 128:(f + 1) * 128, :])
                for i, tl in enumerate(tls):
                    for n in range(2):
                        nc.tensor.matmul(
                            pso[i * 2 + n][:],
                            m1T[:, f, tl * 128:(tl + 1) * 128],
                            wo[:, n * 512:(n + 1) * 512],
                            start=False, stop=(f == NF - 1))

            def fc2_drain(pso, tls):
                for i, tl in enumerate(tls):
                    for n in range(2):
                        ot = p4o.tile([128, 512], f32, name="ot", tag="ot")
                        nc.vector.tensor_tensor(
                            ot[:], pso[i * 2 + n][:], xP[:, tl, n, :],
                            OP.add)
                        nc.sync.dma_start(
                            out_d[tl * 128:(tl + 1) * 128,
                                  n * 512:(n + 1) * 512], ot[:])

            # phase A: fc1 on token cols 0:384 (overlaps the final RS),
            # fc2 on tokens 0:256, offset by one f so the relu drain of
            # fc1(f) never stalls the fc2 matmuls
            pso_a = fc2_alloc_init((0, 1))
            for f in range(NF):
                emit_fc1_f(f, 0, 384)
                if f >= 1:
                    emit_fc2_f(pso_a, f - 1, (0, 1))
                if f == 8:
                    emit_resid_ln2(3)
            emit_fc2_f(pso_a, NF - 1, (0, 1))
            fc2_drain(pso_a, (0, 1))

            # phase B: fc1 on cols 384:512, fc2 on tokens 256:512
            pso_b = fc2_alloc_init((2, 3))
            for f in range(NF):
                emit_fc1_f(f, 384, 512)
                if f >= 1:
                    emit_fc2_f(pso_b, f - 1, (2, 3))
            emit_fc2_f(pso_b, NF - 1, (2, 3))
            fc2_drain(pso_b, (2, 3))
    nc.compile()
    return nc


def own_token_idx(t, S=S_FULL, GROUP=GROUP_FULL):
    CSG = CS // GROUP
    return np.concatenate([
        np.arange(qc * CS + t * CSG, qc * CS + (t + 1) * CSG)
        for qc in range(S // CS)])


def marshal_inputs(x, ln1_g, ln1_b, ln2_g, ln2_b, W_qkv, b_qkv, W_proj,
                   b_proj, W_fc, b_fc, W_out, b_out,
                   S=S_FULL, DFF=DFF_FULL, GROUP=GROUP_FULL,
                   n_cores=N_CORES):
    NF = DFF // 128
    import ml_dtypes
    adt = ml_dtypes.bfloat16
    mdt = ml_dtypes.bfloat16

    def f32c(a):
        return np.ascontiguousarray(a, dtype=np.float32)

    def ac(a):
        return np.ascontiguousarray(a, dtype=adt)

    def mc(a):
        return np.ascontiguousarray(a, dtype=mdt)

    # fold LN1 gamma/beta into W_qkv / b_qkv
    W_qkv_f = W_qkv * ln1_g[:, None]
    b_qkv_f = ln1_b @ W_qkv + b_qkv
    # fold LN2 gamma/beta into W_fc / b_fc
    W_fc_f = W_fc * ln2_g[:, None]
    b_fc_f = ln2_b @ W_fc + b_fc
    # fold the V bias through proj (softmax rows sum to 1), /4 because the
    # ReduceScatter sums 4 per-core partials each carrying bias/4
    bv_full = b_qkv_f[2 * D:3 * D]
    bprojq = (b_proj + bv_full @ W_proj) * 0.25

    base = {
        "bfc_m": f32c(b_fc_f.reshape(NF, 128).T),
        "wfc_m": mc(W_fc_f.reshape(DJ, 128, NF, 128).transpose(2, 1, 0, 3)),
        "wout_m": mc(W_out),
        "bprojq_m": f32c(bprojq.reshape(1, D)),
        "bout_m": mc(b_out.reshape(1, D)),
    }
    in_maps = []
    for c in range(n_cores):
        g, t = c // GROUP, c % GROUP
        cs, ce = t * 256, (t + 1) * 256
        wq = W_qkv_f[:, cs:ce]
        wk = W_qkv_f[:, D + cs:D + ce]
        wv = W_qkv_f[:, 2 * D + cs:2 * D + ce]
        bq = b_qkv_f[cs:ce]
        bk = b_qkv_f[D + cs:D + ce]
        wp = W_proj[cs:ce, :]
        m = dict(base)
        m["x_b"] = f32c(x[g])
        m["x_own"] = f32c(x[g][own_token_idx(t, S, GROUP)])
        m["wq_m"] = ac(wq.reshape(DJ, 128, 256).transpose(1, 0, 2))
        m["wk_m"] = ac(wk.reshape(DJ, 128, 256).transpose(1, 0, 2))
        m["wv_m"] = ac(wv.reshape(DJ, 128, 256).transpose(1, 0, 2))
        m["bq_m"] = f32c(bq.reshape(2, 128).T)
        m["bk_m"] = f32c(bk.reshape(2, 128).T)
        m["wproj_m"] = ac(
            wp.reshape(2, 2, 64, D).transpose(1, 2, 0, 3).reshape(128, 2, D))
        in_maps.append(m)
    return in_maps


_NC_CACHE = {}


def _get_nc():
    if "nc" not in _NC_CACHE:
        _NC_CACHE["nc"] = build_nc()
    return _NC_CACHE["nc"]


def kernel(**inputs):
    inputs = {k: np.asarray(v, dtype=np.float32) for k, v in inputs.items()}
    nc = _get_nc()
    in_maps = marshal_inputs(**inputs)
    r = run_bass_kernel_spmd(nc, in_maps, core_ids=list(range(N_CORES)))
    out = np.empty((B, S_FULL, D), np.float32)
    for c in range(N_CORES):
        g, t = c // GROUP_FULL, c % GROUP_FULL
        out[g, own_token_idx(t), :] = r.results[c]["out_s"]
    return out
